# revision 13
# baseline (speedup 1.0000x reference)
"""Trainium2 Bass kernel for nn_ContrastiveLoss (B=4, C=256, H=W=256).

Strategy
--------
The reference computes four families of per-position channel dot products
over columns of x viewed as [B, C, N] (N = H*W), then scalar reductions:

  fam1 (pos_sim): dot(x[:,:,pos[t]],  x[:,:,pos[t+P]])   t in [0,P)
  fam2 (neg_sim): dot(x[:,:,neg[t]],  x[:,:,neg[t+Ng]])  t in [0,Ng)
  fam3 (pn1):     dot(x[:,:,pos[t]],  x[:,:,neg[t]])     t in [0,M)
  fam4 (pn2):     dot(x[:,:,pos[t]],  x[:,:,neg[t]])     t in [M,2M)

Each column of x participates in at most two dot products, so the union of
the four pairings is a degree-<=2 graph = disjoint paths and even cycles.
The host walks those paths/cycles and emits columns in walk order; in the
permuted tensor xp every dot product is between ADJACENT columns.  The
device then only streams xp once (the HBM roofline), computes shifted
products xp[:,:,i]*xp[:,:,i+1], reduces over C via a ones-vector matmul on
the tensor engine (PSUM-accumulated over the two 128-channel chunks), and
applies per-family 0/1 masks to form the four partial reductions.  Cycles
are closed by re-emitting their first column; junk edges between components
and in padding have all-zero masks.

Sharding: the edge list is split into 8 equal contiguous chunks of the
column walk (one per NeuronCore, overlapping by one column).  Each core
returns 4 partial scalars (sum d*m1, sum d*m2, sum exp(d)*m3, sum
exp(d)*m4); the host combines them into the final loss.  exp() needs no
max-subtraction: |d| < ~0.5 for this data regime, so sum(exp(d)) is stable
in fp32 (guarded by an assert on the host).
"""

import math
import sys

import numpy as np

if "/opt/trn_rl_repo" not in sys.path:  # harness runs from a fresh dir
    sys.path.insert(0, "/opt/trn_rl_repo")

B, C, N = 4, 256, 65536
N_CORES = 8
BLOCK = 512          # edges per PSUM block (= max fp32 matmul free dim)
CHUNKS = C // 128    # channel chunks of 128 partitions


# ---------------------------------------------------------------- host prep

def _build_walk(y):
    """Column permutation + per-edge family labels (0 = junk/padding)."""
    y = np.asarray(y).reshape(-1)
    pos_idx = np.nonzero(y == 1)[0]
    neg_idx = np.nonzero(y == 0)[0]
    P = pos_idx.shape[0] // 2
    Ng = neg_idx.shape[0] // 2
    M = min(P, Ng)

    nP, nN = 2 * P, 2 * Ng
    V = nP + nN
    t_pos = np.arange(nP)
    t_neg = np.arange(nN)
    nbrA = np.empty(V, dtype=np.int64)
    nbrA[:nP] = np.where(t_pos < P, t_pos + P, t_pos - P)
    nbrA[nP:] = nP + np.where(t_neg < Ng, t_neg + Ng, t_neg - Ng)
    famA = np.empty(V, dtype=np.int8)
    famA[:nP] = 1
    famA[nP:] = 2
    nbrB = np.full(V, -1, dtype=np.int64)
    nbrB[:2 * M] = nP + t_pos[:2 * M]
    nbrB[nP:nP + 2 * M] = t_neg[:2 * M]
    famB = np.zeros(V, dtype=np.int8)
    famB[:M] = 3
    famB[M:2 * M] = 4
    famB[nP:nP + M] = 3
    famB[nP + M:nP + 2 * M] = 4

    visited = np.zeros(V, dtype=bool)
    order = np.empty(V + V // 4 + 16, dtype=np.int64)
    fams_l = np.empty(order.shape[0], dtype=np.int8)
    no = 0
    ne = 0

    def walk_from(v0, is_cycle):
        nonlocal no, ne
        if no > 0:
            fams_l[ne] = 0  # junk edge joining the previous component
            ne += 1
        v = v0
        use_A = True  # endpoints/cycle starts leave via their A edge
        order[no] = v
        no += 1
        visited[v] = True
        while True:
            if use_A:
                nxt, fam = nbrA[v], famA[v]
            else:
                nxt = nbrB[v]
                if nxt < 0:
                    return
                fam = famB[v]
            if visited[nxt]:
                if is_cycle and nxt == v0 and not use_A:
                    fams_l[ne] = fam
                    ne += 1
                    order[no] = v0  # close the cycle
                    no += 1
                return
            fams_l[ne] = fam
            ne += 1
            order[no] = nxt
            no += 1
            visited[nxt] = True
            v = nxt
            use_A = not use_A

    for v0 in np.nonzero(nbrB < 0)[0]:
        if not visited[v0]:
            walk_from(int(v0), is_cycle=False)
    for v0 in range(V):
        if not visited[v0]:
            walk_from(int(v0), is_cycle=True)

    n_real = int((fams_l[:ne] > 0).sum())
    assert n_real == P + Ng + 2 * M, (n_real, P + Ng + 2 * M)

    per = N_CORES * BLOCK
    E_pad = ((ne + per - 1) // per) * per
    V_pad = E_pad + 1
    fams = np.zeros(E_pad, dtype=np.int8)
    fams[:ne] = fams_l[:ne]
    vert = np.zeros(V_pad, dtype=np.int64)
    vert[:no] = order[:no]
    colmap = np.where(vert < nP, pos_idx[np.minimum(vert, nP - 1)],
                      neg_idx[np.maximum(vert - nP, 0)])
    return colmap, fams, P, Ng, M


# ------------------------------------------------------------- device program

def trace_program(nc, tc, ctx, S, nb, dt_in):
    """Emit the per-core program. S = edges/core, nb = S//BLOCK.

    DRAM tensors (per core): xp [B, C, S+1] dt_in, msk [4, 4*nb, BLOCK] f32,
    out [1, 4] f32 = (sum d*m1, sum d*m2, sum exp(d)*m3, sum exp(d)*m4).
    """
    import concourse.mybir as mybir

    f32 = mybir.dt.float32
    R = 4 * nb
    xp = nc.dram_tensor("xp", [B, C, S + 1], dt_in, kind="ExternalInput").ap()
    msk = nc.dram_tensor("msk", [4, R, BLOCK], f32, kind="ExternalInput").ap()
    out = nc.dram_tensor("out", [1, 4], f32, kind="ExternalOutput").ap()
    trace_program_aps(nc, tc, ctx, S, nb, dt_in, xp, msk, out)


def trace_program_aps(nc, tc, ctx, S, nb, dt_in, xp, msk, out,
                      gpsimd_on=(2, 5, 7), xp_bufs=10, prod_bufs=6):
    import concourse.mybir as mybir

    f32 = mybir.dt.float32
    R = 4 * nb  # d rows: block k, batch b -> row 4k+b

    const_pool = ctx.enter_context(tc.tile_pool(name="const", bufs=1))
    mask_pool = ctx.enter_context(tc.tile_pool(name="masks", bufs=1))
    xp_pool = ctx.enter_context(tc.tile_pool(name="xp", bufs=xp_bufs))
    prod_pool = ctx.enter_context(tc.tile_pool(name="prod", bufs=prod_bufs))
    stat_pool = ctx.enter_context(tc.tile_pool(name="stat", bufs=1))
    psum_pool = ctx.enter_context(tc.tile_pool(name="psum", bufs=1, space="PSUM"))

    # Staircase selector: zo[:, 63] = 1, else 0.  lhsT = zo[:, 63-r : 127-r]
    # is a [128, R] one-hot-column matrix that routes a partition-dim
    # column-sum into PSUM row r (other rows accumulate exact zeros) --
    # matmul PSUM outputs must start at partition 0/32/64, so rows can't be
    # addressed via the output AP.
    dt_prod = dt_in  # DVE converts on write for free; PE runs 16-bit at rate
    zo = const_pool.tile([128, 63 + R], dt_prod)
    nc.vector.memset(zo[:], 0.0)
    nc.vector.memset(zo[:, 63:64], 1.0)
    ones_f32 = const_pool.tile([128, 1], f32)
    nc.vector.memset(ones_f32[:], 1.0)

    m_tiles = []
    for f in range(4):
        mt = mask_pool.tile([R, BLOCK], f32, tag=f"m{f}")
        nc.sync.dma_start(mt[:], msk[f])
        m_tiles.append(mt)

    d_psum = psum_pool.tile([R, BLOCK], f32)

    # KB 512-edge blocks per DMA/mul tile: fewer, larger DMAs and DVE ops
    KB = 4 if nb % 4 == 0 else (2 if nb % 2 == 0 else 1)
    W = KB * BLOCK
    n_mm = nb * B * CHUNKS
    i_mm = 0
    mul_i = 0
    for kb in range(nb // KB):
        for b in range(B):
            prods = []
            for c in range(CHUNKS):
                t = xp_pool.tile([128, W + 1], dt_in)
                nc.sync.dma_start(
                    t[:], xp[b, 128 * c:128 * (c + 1), W * kb:W * (kb + 1) + 1])
                p = prod_pool.tile([128, W], dt_prod)
                # DVE is the mul bottleneck; GpSimd runs these ~1.7x
                # slower, so balance ~3/8 of them onto it
                eng = nc.gpsimd if (gpsimd_on and
                                    mul_i % 8 in gpsimd_on) else nc.vector
                eng.tensor_mul(p[:], t[:, 0:W], t[:, 1:W + 1])
                mul_i += 1
                prods.append(p)
            for j in range(KB):
                row = 4 * (kb * KB + j) + b
                for c in range(CHUNKS):
                    nc.tensor.matmul(
                        d_psum[:, :], zo[:, 63 - row:63 - row + R],
                        prods[c][:, BLOCK * j:BLOCK * (j + 1)],
                        start=(i_mm == 0), stop=(i_mm == n_mm - 1))
                    i_mm += 1

    exp_sb = stat_pool.tile([R, BLOCK], f32)
    nc.scalar.activation(exp_sb[:], d_psum[:],
                         mybir.ActivationFunctionType.Exp)

    rcat = stat_pool.tile([R, 4], f32)
    srcs = [d_psum, d_psum, exp_sb, exp_sb]
    for f in range(4):
        scratch = stat_pool.tile([R, BLOCK], f32, tag="scr")
        nc.vector.tensor_mul(scratch[:], srcs[f][:], m_tiles[f][:])
        nc.vector.reduce_sum(rcat[:, f:f + 1], scratch[:],
                             axis=mybir.AxisListType.X)

    f_psum = psum_pool.tile([1, 4], f32, tag="final")
    nc.tensor.matmul(f_psum[:], ones_f32[0:R, :], rcat[:], start=True, stop=True)
    res = stat_pool.tile([1, 4], f32)
    nc.scalar.copy(res[:], f_psum[:])
    nc.sync.dma_start(out, res[:])


_CACHE = {}


def _compiled(S, nb, dt_name):
    key = (S, nb, dt_name)
    if key in _CACHE:
        return _CACHE[key]
    from contextlib import ExitStack

    import concourse.bacc as bacc
    import concourse.mybir as mybir
    import concourse.tile as tile

    dt_in = getattr(mybir.dt, dt_name)
    nc = bacc.Bacc("TRN2", target_bir_lowering=False, debug=False,
                   num_devices=N_CORES)
    with tile.TileContext(nc) as tc:
        with ExitStack() as ctx:
            trace_program(nc, tc, ctx, S, nb, dt_in)
    nc.compile()
    _CACHE[key] = nc
    return nc


# -------------------------------------------------------------------- kernel

def kernel(x, y, _dt_name="float32", _run_opts=None):
    x = np.asarray(x)
    y = np.asarray(y)
    assert x.shape == (B, C, 256, 256) and y.shape == (N,)

    colmap, fams, P, Ng, M = _build_walk(y)
    E = fams.shape[0]
    S = E // N_CORES
    nb = S // BLOCK
    assert nb * BLOCK * N_CORES == E and 4 * nb <= 128

    np_dt = {"float32": np.float32, "float16": np.float16}[_dt_name]
    x3 = x.reshape(B, C, N)
    xp = np.ascontiguousarray(x3[:, :, colmap], dtype=np_dt)  # [B, C, E+1]

    # masks in d-row layout: row 4k+b of core i covers edges
    # i*S + k*BLOCK + [0, BLOCK), identical for the 4 b rows
    fams_c = fams.reshape(N_CORES, nb, 1, BLOCK)
    m = np.empty((N_CORES, 4, 4 * nb, BLOCK), dtype=np.float32)
    for f in range(4):
        m[:, f] = np.broadcast_to(fams_c == f + 1,
                                  (N_CORES, nb, 4, BLOCK)
                                  ).reshape(N_CORES, 4 * nb, BLOCK)

    in_maps = [
        {"xp": np.ascontiguousarray(xp[:, :, i * S:(i + 1) * S + 1]),
         "msk": m[i]}
        for i in range(N_CORES)
    ]

    nc = _compiled(S, nb, _dt_name)
    from concourse.bass_utils import run_bass_kernel_spmd

    res = run_bass_kernel_spmd(nc, in_maps, list(range(N_CORES)),
                               **(_run_opts or {}))
    partials = np.stack([r["out"][0] for r in res.results])  # [N_CORES, 4]
    s1, s2, s3, s4 = partials.sum(axis=0, dtype=np.float64)

    n = float(B * M)
    loss = (-s1 / (B * P) - s2 / (B * Ng)
            + math.log(s3) - math.log(n) + math.log(s4) - math.log(n))
    assert np.isfinite(loss)
    out = np.float32(loss)
    if _run_opts:
        return out, res
    return out
